# revision 10
# baseline (speedup 1.0000x reference)
"""Trainium2 Bass kernel for per-sample modulated causal Conv3D.

Reference semantics (see problem):
  w[b,o,i,kt,kh,kw] = W[o,i,kt,kh,kw] * (cond[b,i] + 1)
  w /= sqrt(max(sum_{i,kt,kh,kw} w^2, eps))        (per b,o)
  y[b] = conv3d(pad(fmap[b], t:(2,0), h:(1,1), w:(1,1)), w[b])  VALID

Shapes: fmap [4,128,16,64,64] f32, cond [4,128], W [128,128,3,3,3].

Strategy: weight modulation/demodulation is a tiny elementwise computation
done on host in float64. The conv runs on 8 NeuronCores, sharded
(batch b) x (time half), 8 output frames per core.

The conv uses Winograd F(4,3) along H only (direct taps in T and W):
per input frame, 6 transformed row-arrays V_p[16 tiles, 66 cols] replace
the 3 kh taps; per output frame the PE accumulates 3kt x 3kw matmuls per
point p into PSUM (M_p), and the output rows are A^T M (4 rows per tile).
PE column-passes drop from 27 to 13.5 per output element pair
(54 matmuls x N=512 per frame vs 108 direct), a 2x PE reduction.

Precision (validated on host + hw): x uploaded bf16; V computed on
DVE/GpSimd with fp32 intermediates, rounded once to bf16; stationary
U = G.W in bf16 (the compiler forbids mixed 16/32-bit matmul inputs);
PSUM fp32; inverse transform in fp32 with one rounding into the bf16
output tile; y downloaded bf16 and upcast on host. Measured rel err:
1.27e-2 host model, 1.43e-2 on hardware (gate 2e-2). Keeping the inverse
intermediates in fp32 is essential: the M_p partially cancel, and bf16
roundings there measured >2e-2.

Engine split per frame: PE 54 matmuls (bound, ~23us); DVE 7 transform ops +
8 inverse ops; GpSimd 5 transform ops + 2 inverse ops; Act engine does the
PSUM pre-copies (m1, m3) so no vector op reads two PSUM operands.
"""

import numpy as np
import orjson

import concourse.bass as bass
import concourse.mybir as mybir
import concourse.tile as tile
from concourse.bass_utils import run_bass_kernel_spmd
from concourse.vector_clock import ScopedClock, VectorClock

# Problem constants (hardcoded per harness contract).
B = 4
DIM = 128
DIM_OUT = 128
T = 16
H = 64
W = 64
TK = 3
SK = 3
EPS = 1e-8
N_CORES = 8
TH = T // 2  # frames per core (time half)
PF = TH + TK - 1  # padded frames per core
HP = H + 2  # spatially padded rows
WP = W + 2  # spatially padded cols
NPT = 6  # Winograd F(4,3) points along h
NTR = 16  # h tiles per frame (4 output rows each)
NTAPS = TK * NPT * SK  # stationary weight count (kt, p, kw)
HW = W // 2  # half-frame column count per matmul (N=512)

DT_MOV = mybir.dt.bfloat16  # V / moving operand dtype
DT_STAT = mybir.dt.bfloat16  # U / stationary operand dtype (the compiler
# rejects mixed 32/16-bit matmul inputs, so U is bf16 like V; measured
# rel err with both bf16 is 1.27e-2 vs the 2e-2 gate)
F32 = mybir.dt.float32
ADD = mybir.AluOpType.add
SUB = mybir.AluOpType.subtract
MULT = mybir.AluOpType.mult

# F(4,3) weight transform (Lavin, correlation form): U_p = sum_kh G[p,kh] w[kh]
G_MAT = np.array(
    [
        [1 / 4, 0, 0],
        [-1 / 6, -1 / 6, -1 / 6],
        [-1 / 6, 1 / 6, -1 / 6],
        [1 / 24, 1 / 12, 1 / 6],
        [1 / 24, -1 / 12, 1 / 6],
        [0, 0, 1],
    ],
    dtype=np.float64,
)

# PSUM drain order: inverse consumes m1,m3 (act copies) then m2,m4,m0,m5,
# so allocate/fill next half's banks in the same order to minimize stalls.
P_ORDER = (1, 3, 2, 4, 0, 5)


class _TileContextChunkedDrain(tile.TileContext):
    """TileContext whose tail drain splits its sem waits across a chain of
    sync NOPs (1 wait each): the walrus build in this container rejects CTRL
    instructions carrying more than one sync-wait command."""

    def _drain_and_barrier(self, tick_clock, wait_clock):
        vec = list(tick_clock.global_clock)
        for i, t in enumerate(vec):
            if t <= 0:
                continue
            v = [0] * len(vec)
            v[i] = t
            nop = self.nc.sync.nop()
            wait_clock.add_sem_waits(nop.ins, ScopedClock({None: VectorClock(v)}))
        self.nc.sync.drain()
        self.nc.all_engine_barrier()
        assert self.sems is not None
        popped = self.nc._tile_sem_poison_stack.pop()
        assert popped is self._sem_poison
        self.nc.clear_and_free_semaphores(list(self.sems.allocated().values()))
        self.nc.all_engine_barrier()


def _split_multi_waits(bir: bytes) -> bytes:
    """The walrus build here rejects instructions carrying more than one
    sync-wait command. Un-fuse: move extra waits onto NoOps inserted just
    before the instruction on the same engine queue (strictly equivalent —
    the engine queue stalls on the NoOp's wait first)."""
    m = orjson.loads(bir)
    ctr = 0
    for f in m["functions"]:
        for bb in f["blocks"]:
            out = []
            for inst in bb["instructions"]:
                si = inst.get("sync_info")
                waits = (si or {}).get("on_wait") or []
                if len(waits) > 1:
                    for w in waits[:-1]:
                        ctr += 1
                        out.append({
                            "debug": inst.get("debug", 0),
                            "engine": inst["engine"],
                            "ins": [],
                            "outs": [],
                            "name": f"I-wsplit{ctr}",
                            "opcode": "NoOp",
                            "sync_info": {"on_update": [], "on_wait": [w]},
                        })
                    si["on_wait"] = [waits[-1]]
                out.append(inst)
            bb["instructions"] = out
    return orjson.dumps(m)


def _patch_to_json_bytes(nc):
    orig = nc.to_json_bytes

    def to_json_bytes():
        return _split_multi_waits(orig())

    nc.to_json_bytes = to_json_bytes
    return nc


def build_nc(repeats=1, warmup_mms=16, xbufs=3, vbufs=4):
    """Build the per-core SPMD Bass program (identical on all cores).

    repeats>1 re-runs the whole body (loads + conv + stores) that many
    times — used only for slope-based HW timing."""
    nc = bass.Bass("TRN2", target_bir_lowering=False, debug=False,
                   num_devices=N_CORES)
    xp = nc.dram_tensor("xp", [DIM, PF, HP, WP], DT_MOV, kind="ExternalInput")
    wm = nc.dram_tensor("wm", [DIM, NTAPS, DIM_OUT], DT_STAT,
                        kind="ExternalInput")
    y = nc.dram_tensor("y", [DIM_OUT, TH, H, W], DT_MOV, kind="ExternalOutput")

    with _TileContextChunkedDrain(nc) as tc:
        with (
            tc.tile_pool(name="wpool", bufs=1) as wpool,
            tc.tile_pool(name="xpool", bufs=xbufs) as xpool,
            tc.tile_pool(name="vpool", bufs=vbufs) as vpool,
            tc.tile_pool(name="spool", bufs=2) as spool,
            tc.tile_pool(name="ipool", bufs=2) as ipool,
            tc.tile_pool(name="ypool", bufs=2) as ypool,
            tc.tile_pool(name="ppool", bufs=8, space="PSUM") as ppool,
        ):
            if warmup_mms:
                # Keep the PE busy during the initial DMA/transform window so
                # the HAM clock gate reaches full p-state before real matmuls.
                wu_w = wpool.tile([DIM, DIM_OUT], DT_STAT, name="wu_w")
                wu_x = wpool.tile([DIM, 512], DT_MOV, name="wu_x")
                nc.gpsimd.memset(wu_w[:], 0.0)
                nc.gpsimd.memset(wu_x[:], 0.0)
                wu_ps = ppool.tile([DIM_OUT, 512], F32, name="wu_ps", tag="psum")
                for _ in range(warmup_mms):
                    nc.tensor.matmul(wu_ps[:], wu_w[:], wu_x[:],
                                     start=True, stop=True)

            wt = wpool.tile([DIM, NTAPS, DIM_OUT], DT_STAT)

            frames = {}
            vmap = {}

            def load_frame(fi, rep=0, chunks=1):
                xt = xpool.tile([DIM, HP, WP], DT_MOV, name=f"x_r{rep}f{fi}",
                                tag="xframe")
                if chunks == 1:
                    nc.sync.dma_start(xt[:], xp.ap()[:, fi])
                else:
                    step = (HP + chunks - 1) // chunks
                    for r0 in range(0, HP, step):
                        r1 = min(r0 + step, HP)
                        nc.sync.dma_start(xt[:, r0:r1], xp.ap()[:, fi, r0:r1])
                frames[fi] = xt

            def transform(fi, rep=0):
                """h-axis F(4,3) input transform of frame fi -> 6 V tiles.

                rows d_j = x[4t+j]; fp32 intermediates, one bf16 rounding:
                  c=d4-d2  k=d5-d3  e=d3-d1  h=d0-d2  a=d4-4d2  b=d3-4d1
                  V0=4h+c  V1=a+b  V2=a-b  V3=c+2e  V4=c-2e  V5=k-4e
                """
                xt = frames.pop(fi)

                def xr(j):
                    return xt[:, j:j + 61:4, :]

                sh = [DIM, NTR, WP]
                c = spool.tile(sh, F32, tag="c")
                k = spool.tile(sh, F32, tag="k")
                e = spool.tile(sh, F32, tag="e")
                h = spool.tile(sh, F32, tag="h")
                a = spool.tile(sh, F32, tag="a")
                b = spool.tile(sh, F32, tag="b")
                vt = [vpool.tile(sh, DT_MOV, name=f"v_r{rep}f{fi}p{p}",
                                 tag=f"v{p}") for p in range(NPT)]
                # TensorScalarPtr (scalar_tensor_tensor) is DVE-only on this
                # core version; GpSimd takes plain tensor_tensor adds.
                nc.gpsimd.tensor_tensor(c[:], xr(4), xr(2), SUB)
                nc.gpsimd.tensor_tensor(k[:], xr(5), xr(3), SUB)
                nc.gpsimd.tensor_tensor(e[:], xr(3), xr(1), SUB)
                nc.vector.tensor_tensor(h[:], xr(0), xr(2), SUB)
                nc.vector.scalar_tensor_tensor(a[:], xr(2), -4.0, xr(4),
                                               MULT, ADD)
                nc.vector.scalar_tensor_tensor(b[:], xr(1), -4.0, xr(3),
                                               MULT, ADD)
                nc.vector.scalar_tensor_tensor(vt[0][:], h[:], 4.0, c[:],
                                               MULT, ADD)
                nc.gpsimd.tensor_tensor(vt[1][:], a[:], b[:], ADD)
                nc.gpsimd.tensor_tensor(vt[2][:], a[:], b[:], SUB)
                nc.vector.scalar_tensor_tensor(vt[3][:], e[:], 2.0, c[:],
                                               MULT, ADD)
                nc.vector.scalar_tensor_tensor(vt[4][:], e[:], -2.0, c[:],
                                               MULT, ADD)
                nc.vector.scalar_tensor_tensor(vt[5][:], e[:], -4.0, k[:],
                                               MULT, ADD)
                vmap[fi] = vt

            # startup: weights + first frames, chunked across DMA queues
            xt0 = xpool.tile([DIM, HP, WP], DT_MOV, name="x_r0f0",
                             tag="xframe")
            frames[0] = xt0
            for cidx in range(3):
                ws = slice(cidx * NPT, (cidx + 1) * NPT)
                nc.sync.dma_start(wt[:, ws], wm.ap()[:, ws])
                r0, r1 = cidx * 22, (cidx + 1) * 22
                nc.sync.dma_start(xt0[:, r0:r1], xp.ap()[:, 0, r0:r1])
            for s0 in range(3 * NPT, NTAPS, NPT):
                nc.sync.dma_start(wt[:, s0:s0 + NPT], wm.ap()[:, s0:s0 + NPT])

            for rep in range(repeats):
                if rep > 0:
                    load_frame(0, rep)
                load_frame(1, rep, chunks=3 if rep == 0 else 1)
                load_frame(2, rep, chunks=3 if rep == 0 else 1)
                transform(0, rep)
                transform(1, rep)
                transform(2, rep)

                for f in range(TH):
                    if f + TK <= PF - 1:
                        load_frame(f + TK, rep)
                        transform(f + TK, rep)
                    yt = ypool.tile([DIM_OUT, H, W], DT_MOV, tag="yframe")
                    for half in range(2):
                        c0 = half * HW
                        ps = {}
                        for p in P_ORDER:
                            ps[p] = ppool.tile([DIM_OUT, NTR, HW], F32,
                                               name=f"ps_r{rep}f{f}h{half}p{p}",
                                               tag="psum")
                            for kt in range(TK):
                                vtile = vmap[f + kt][p]
                                for kw in range(SK):
                                    nc.tensor.matmul(
                                        ps[p][:],
                                        wt[:, kt * NPT * SK + p * SK + kw],
                                        vtile[:, :, c0 + kw:c0 + kw + HW],
                                        start=(kt == 0 and kw == 0),
                                        stop=(kt == TK - 1 and kw == SK - 1),
                                    )
                        # GpSimd has no PSUM port: Act stages m1/m3/m4 into
                        # SBUF so GpSimd can compute e_, f_, y0 there.
                        ish = [DIM_OUT, NTR, HW]
                        m1c = ipool.tile(ish, F32, tag="m1c")
                        m3c = ipool.tile(ish, F32, tag="m3c")
                        m4c = ipool.tile(ish, F32, tag="m4c")
                        nc.scalar.copy(m1c[:], ps[1][:])
                        nc.scalar.copy(m3c[:], ps[3][:])
                        nc.scalar.copy(m4c[:], ps[4][:])
                        s_ = ipool.tile(ish, F32, tag="s")
                        d_ = ipool.tile(ish, F32, tag="d")
                        e_ = ipool.tile(ish, F32, tag="e2")
                        f_ = ipool.tile(ish, F32, tag="f2")
                        t1 = ipool.tile(ish, F32, tag="t1")
                        t2 = ipool.tile(ish, F32, tag="t2")
                        nc.vector.tensor_tensor(s_[:], m1c[:], ps[2][:], ADD)
                        nc.vector.tensor_tensor(d_[:], m1c[:], ps[2][:], SUB)
                        nc.gpsimd.tensor_tensor(e_[:], m3c[:], m4c[:], ADD)
                        nc.gpsimd.tensor_tensor(f_[:], m3c[:], m4c[:], SUB)
                        nc.vector.tensor_tensor(t1[:], ps[0][:], s_[:], ADD)

                        def yrow(i):
                            return yt[:, i:i + 61:4, c0:c0 + HW]

                        nc.gpsimd.tensor_tensor(yrow(0), t1[:], e_[:], ADD)
                        nc.vector.scalar_tensor_tensor(yrow(1), f_[:], 2.0,
                                                       d_[:], MULT, ADD)
                        nc.vector.scalar_tensor_tensor(yrow(2), e_[:], 4.0,
                                                       s_[:], MULT, ADD)
                        nc.vector.scalar_tensor_tensor(t2[:], f_[:], 8.0,
                                                       d_[:], MULT, ADD)
                        nc.vector.tensor_tensor(yrow(3), t2[:], ps[5][:], ADD)
                    del vmap[f]
                    # store per half-row-group so the tail only waits on the
                    # last rows for the final frame
                    last = (rep == repeats - 1 and f == TH - 1)
                    if last:
                        for r0 in range(0, H, 16):
                            nc.sync.dma_start(y.ap()[:, f, r0:r0 + 16],
                                              yt[:, r0:r0 + 16])
                    else:
                        nc.sync.dma_start(y.ap()[:, f], yt[:])
                vmap.clear()
                frames.clear()
    return _patch_to_json_bytes(nc)


def modulate_weights(weights, cond):
    """Host-side weight modulation + demodulation (float64)."""
    w = weights.astype(np.float64)[None] * (cond.astype(np.float64)[:, None, :,
                                                                    None, None,
                                                                    None] + 1.0)
    ssq = np.sum(w * w, axis=(2, 3, 4, 5), keepdims=True)
    w = w / np.sqrt(np.maximum(ssq, EPS))
    return w  # [B, O, I, TK, SK, SK] float64


def prepare_inputs(fmap, cond, weights):
    """Shard full inputs into per-core input maps."""
    import ml_dtypes

    fmap = np.asarray(fmap, dtype=np.float32)
    cond = np.asarray(cond, dtype=np.float32)
    weights = np.asarray(weights, dtype=np.float32)

    wmod = modulate_weights(weights, cond)
    # U[b, i, kt, p, kw, o] = sum_kh G[p, kh] * wmod[b, o, i, kt, kh, kw]
    u = np.einsum('ph,boithw->bitpwo', G_MAT, wmod)
    u = np.ascontiguousarray(
        u.reshape(B, DIM, NTAPS, DIM_OUT)).astype(ml_dtypes.bfloat16)

    xpad = np.zeros((B, DIM, T + TK - 1, HP, WP), np.float32)
    xpad[:, :, TK - 1:, 1:1 + H, 1:1 + W] = fmap
    xpad = xpad.astype(ml_dtypes.bfloat16)

    in_maps = []
    for c in range(N_CORES):
        b, th = divmod(c, 2)
        t0 = th * TH
        in_maps.append({
            "xp": np.ascontiguousarray(xpad[b, :, t0:t0 + PF]),
            "wm": u[b],
        })
    return in_maps


def assemble_output(results):
    """Gather per-core bf16 [O, TH, H, W] results into f32 [B, O, T, H, W]."""
    out = np.empty((B, DIM_OUT, T, H, W), np.float32)
    for c, r in enumerate(results):
        b, th = divmod(c, 2)
        out[b, :, th * TH:(th + 1) * TH] = np.asarray(r["y"],
                                                      dtype=np.float32)
    return out


_NC_CACHE = []


def kernel(**inputs):
    fmap = inputs["fmap"]
    cond = inputs["cond"]
    weights = inputs["weights"]
    in_maps = prepare_inputs(fmap, cond, weights)
    if not _NC_CACHE:
        _NC_CACHE.append(build_nc())
    nc = _NC_CACHE[0]
    res = run_bass_kernel_spmd(nc, in_maps, core_ids=list(range(N_CORES)))
    return assemble_output(res.results)
